# revision 1
# baseline (speedup 1.0000x reference)
"""GCN 2-layer kernel on 8 Trainium2 NeuronCores (Bass/Tile).

Sharding: core m owns dest rows [m*R, (m+1)*R). Edges partitioned by dest
row (core), then by source chunk (8 GPSIMD core-groups per NC), sorted by
dest row. SpMM per layer:
  - table (128, R) in SBUF: partition 16g+f = feature f of node chunk g
    (chunk g == core g's row shard, delivered by partition-axis AllGather)
  - ap_gather (GPSIMD): per-group edge-ordered gather from table
  - DVE: multiply by edge vals, plain prefix scan per partition
  - ap_gather #2: extract prefix values at per-row end positions
  - DVE shifted subtract -> per-(row,group) segment sums
  - TensorE one-hot SEL matmul: sum 8 group-partials -> (feat, rows)
Dense matmuls (X@W1, h@W2) on TensorE; log_softmax via TensorE transpose +
DVE/ACT ops.
"""

import sys

for p in ("/opt/trn_rl_repo",):
    if p not in sys.path:
        sys.path.insert(0, p)

import numpy as np

import concourse.bass as bass
import concourse.mybir as mybir
import concourse.tile as tile
from concourse import bacc, library_config

F32 = mybir.dt.float32
I16 = mybir.dt.int16


class Cfg:
    def __init__(self, N, E, IN, HID, OUT, SUBS, NW):
        self.N = N            # nodes
        self.E = E            # edges
        self.IN = IN          # input feat
        self.HID = HID        # hidden feat (16)
        self.OUT = OUT        # out feat (7)
        self.C = 8            # cores
        self.R = N // 8       # rows per core == nodes per source chunk
        self.SUBS = SUBS      # sub-chunks (row ranges) per core
        self.SUBROWS = self.R // SUBS
        self.EXT = -(-(self.SUBROWS + 1) // 16) * 16  # ext list len (lead + rows, %16)
        self.NW = NW          # column windows for dense matmuls
        self.WCOL = self.R // NW
        assert self.R % SUBS == 0 and self.R % NW == 0 and self.WCOL <= 512
        self.KC = -(-IN // 128)  # K chunks
        self.KLAST = IN - (self.KC - 1) * 128


FULL = Cfg(N=100_000, E=3_200_000, IN=1433, HID=16, OUT=7, SUBS=50, NW=50)


def prepare(x, adj_row, adj_col, adj_val, W1, b1, W2, b2, cfg):
    """Host preprocessing: build per-core input maps (pure numpy)."""
    N, E, R, C = cfg.N, cfg.E, cfg.R, cfg.C
    SUBS, SUBROWS, EXT = cfg.SUBS, cfg.SUBROWS, cfg.EXT

    core = adj_row // R
    grp = adj_col // R
    # sort edges by (core, sub, group, row)
    sub = (adj_row - core * R) // SUBROWS
    order = np.lexsort((adj_row, grp, sub, core))
    r_s = adj_row[order]
    c_s = adj_col[order]
    v_s = adj_val[order]
    core_s = core[order]
    grp_s = grp[order]
    sub_s = sub[order]

    # cell boundaries: (core, sub, group)
    key = (core_s * SUBS + sub_s) * C + grp_s
    ncell = C * SUBS * C
    starts = np.searchsorted(key, np.arange(ncell))
    ends = np.searchsorted(key, np.arange(ncell) + 1)
    cnt = (ends - starts).reshape(C, SUBS, C)

    # global common sub-stream lengths (1 dummy + edges, padded to %16)
    glk = cnt.max(axis=(0, 2)) + 1
    glk = (-(-glk // 16) * 16).astype(np.int64)
    glk = np.maximum(glk, 16)
    gtot = int(glk.sum())
    offs = np.concatenate([[0], np.cumsum(glk)])

    xt = np.ascontiguousarray(x.T)  # (IN, N)
    W1f = np.ascontiguousarray(W1, dtype=np.float32)
    KC = cfg.KC
    w1p = np.zeros((KC * 128, cfg.HID), np.float32)
    w1p[: cfg.IN] = W1f

    sel1 = np.zeros((128, 16), np.float32)
    sel1[np.arange(128), np.arange(128) % 16] = 1.0
    sel2 = np.zeros((128, 16), np.float32)  # use 16 cols, first OUT real
    sel2[np.arange(128), np.arange(128) % 16] = 1.0
    for p in range(128):
        if p % 16 >= cfg.OUT:
            sel2[p, p % 16] = 0.0
    ident = np.eye(128, dtype=np.float32)

    in_maps = []
    for m in range(C):
        gidx = np.zeros((C, gtot), np.int16)          # per-group wrapped later
        valr = np.zeros((C, gtot), np.float32)
        eidx = np.zeros((C, SUBS * EXT), np.int16)
        for k in range(SUBS):
            L = glk[k]
            o = offs[k]
            for g in range(C):
                ci = (m * SUBS + k) * C + g
                s, e = starts[ci], ends[ci]
                n = e - s
                assert n + 1 <= L
                # slot 0 dummy (idx 0, val 0); then edges
                gidx[g, o + 1 : o + 1 + n] = (c_s[s:e] - g * R).astype(np.int16)
                valr[g, o + 1 : o + 1 + n] = v_s[s:e]
                # extraction: lead(pos0) + per-row last-slot position
                rows_rel = r_s[s:e] - (m * R + k * SUBROWS)
                pos = np.searchsorted(rows_rel, np.arange(SUBROWS), side="right")
                ex = np.zeros(EXT, np.int16)
                ex[1 : 1 + SUBROWS] = pos.astype(np.int16)  # pos incl. dummy offset
                ex[1 + SUBROWS :] = pos[-1] if SUBROWS else 0
                eidx[g, k * EXT : (k + 1) * EXT] = ex
        # wrap per-group lists into 16 partitions: idxs[pl, s] = list[s*16+pl]
        def wrap16(a):
            # (C, L) -> (128, L//16): group g occupies partitions 16g..16g+16
            Cg, L = a.shape
            out = np.zeros((128, L // 16), a.dtype)
            for g in range(Cg):
                out[16 * g : 16 * g + 16] = a[g].reshape(-1, 16).T
            return out

        gidx_w = wrap16(gidx)
        eidx_w = wrap16(eidx)
        valrep = np.repeat(valr, 16, axis=0)  # (128, gtot)

        in_maps.append(
            dict(
                xt=np.ascontiguousarray(xt[:, m * R : (m + 1) * R]),
                w1p=w1p,
                b1=np.ascontiguousarray(b1.reshape(cfg.HID, 1), dtype=np.float32),
                w2=np.ascontiguousarray(W2, dtype=np.float32),
                b2=np.ascontiguousarray(b2.reshape(cfg.OUT, 1), dtype=np.float32),
                gidx=gidx_w,
                eidx=eidx_w,
                valrep=valrep,
                sel1=sel1,
                sel2=sel2,
                ident=ident,
            )
        )
    return in_maps, glk, offs


def build(cfg, glk, offs, dbg=False):
    """Build the SPMD Bass program (one program, per-core data via inputs)."""
    nc = bacc.Bacc("TRN2", target_bir_lowering=False, debug=False, num_devices=cfg.C)
    R, HID, OUT, SUBS, EXT, NW, WCOL = (
        cfg.R, cfg.HID, cfg.OUT, cfg.SUBS, cfg.EXT, cfg.NW, cfg.WCOL,
    )
    KC, KLAST = cfg.KC, cfg.KLAST
    gtot = int(glk.sum())
    glkmax = int(max(glk))
    NRT = -(-R // 128)               # row tiles for softmax
    RLAST = R - (NRT - 1) * 128

    xt = nc.dram_tensor("xt", [cfg.IN, R], F32, kind="ExternalInput").ap()
    w1p = nc.dram_tensor("w1p", [KC * 128, HID], F32, kind="ExternalInput").ap()
    b1 = nc.dram_tensor("b1", [HID, 1], F32, kind="ExternalInput").ap()
    w2 = nc.dram_tensor("w2", [HID, OUT], F32, kind="ExternalInput").ap()
    b2 = nc.dram_tensor("b2", [OUT, 1], F32, kind="ExternalInput").ap()
    gidx = nc.dram_tensor("gidx", [128, gtot // 16], I16, kind="ExternalInput").ap()
    eidx = nc.dram_tensor("eidx", [128, SUBS * EXT // 16], I16, kind="ExternalInput").ap()
    valrep = nc.dram_tensor("valrep", [128, gtot], F32, kind="ExternalInput").ap()
    sel1 = nc.dram_tensor("sel1", [128, 16], F32, kind="ExternalInput").ap()
    sel2 = nc.dram_tensor("sel2", [128, 16], F32, kind="ExternalInput").ap()
    ident = nc.dram_tensor("ident", [128, 128], F32, kind="ExternalInput").ap()
    out = nc.dram_tensor("out", [R, OUT], F32, kind="ExternalOutput").ap()
    if dbg:
        dbg_h1t = nc.dram_tensor("dbg_h1t", [HID, R], F32, kind="ExternalOutput").ap()
        dbg_tab = nc.dram_tensor("dbg_tab", [128, R], F32, kind="ExternalOutput").ap()
        dbg_h = nc.dram_tensor("dbg_h", [HID, R], F32, kind="ExternalOutput").ap()

    rg = [list(range(cfg.C))]

    with tile.TileContext(nc) as tc:
        with (
            tc.tile_pool(name="const", bufs=1) as cpool,
            tc.tile_pool(name="tab", bufs=1) as tabpool,
            tc.tile_pool(name="small", bufs=2) as smpool,
            tc.tile_pool(name="soft", bufs=1) as sfpool,
            tc.tile_pool(name="xw", bufs=2) as xpool,
            tc.tile_pool(name="stream", bufs=2) as spool,
            tc.tile_pool(name="psum", bufs=2, space="PSUM") as ppool,
            tc.tile_pool(name="psum2", bufs=2, space="PSUM") as ppool2,
            tc.tile_pool(name="dram", bufs=1, space="DRAM") as dpool,
        ):
            nc.gpsimd.load_library(library_config.ap_gather)

            # ---- consts to SBUF
            w1s = cpool.tile([128, KC, HID], F32)
            nc.sync.dma_start(w1s[:], w1p.rearrange("(k p) m -> p k m", p=128))
            b1s = cpool.tile([HID, 1], F32)
            nc.sync.dma_start(b1s[:], b1[:])
            w2s = cpool.tile([HID, OUT], F32)
            nc.sync.dma_start(w2s[:], w2[:])
            b2s = cpool.tile([OUT, 1], F32)
            nc.sync.dma_start(b2s[:], b2[:])
            sel1s = cpool.tile([128, 16], F32)
            nc.sync.dma_start(sel1s[:], sel1[:])
            sel2s = cpool.tile([128, 16], F32)
            nc.sync.dma_start(sel2s[:], sel2[:])
            idents = cpool.tile([128, 128], F32)
            nc.sync.dma_start(idents[:], ident[:])
            ones = cpool.tile([128, glkmax], F32)
            nc.vector.memset(ones[:], 1.0)

            ag_in1 = dpool.tile([HID, R], F32)
            ag_out1 = dpool.tile([128, R], F32)
            ag_in2 = dpool.tile([16, R], F32)
            ag_out2 = dpool.tile([128, R], F32)
            h_dram = dpool.tile([HID, R], F32)
            zc_dram = dpool.tile([OUT, R], F32)

            # ---- phase A: (X @ W1)^T windows -> ag_in1 (DRAM)
            for w in range(NW):
                xw = xpool.tile([128, KC, WCOL], F32, tag="xw")
                if KC > 1:
                    nc.sync.dma_start(
                        xw[:, : KC - 1, :],
                        xt[: (KC - 1) * 128, w * WCOL : (w + 1) * WCOL].rearrange(
                            "(k p) c -> p k c", p=128
                        ),
                    )
                nc.sync.dma_start(
                    xw[:KLAST, KC - 1, :],
                    xt[(KC - 1) * 128 :, w * WCOL : (w + 1) * WCOL],
                )
                pa = ppool.tile([HID, WCOL], F32, tag="pa")
                for k in range(KC):
                    kp = 128 if k < KC - 1 else KLAST
                    nc.tensor.matmul(
                        pa[:],
                        w1s[:kp, k, :],
                        xw[:kp, k, :],
                        start=(k == 0),
                        stop=(k == KC - 1),
                    )
                st1 = smpool.tile([HID, WCOL], F32, tag="st1")
                nc.scalar.copy(st1[:], pa[:])
                nc.sync.dma_start(ag_in1[:, w * WCOL : (w + 1) * WCOL], st1[:])
                if dbg:
                    nc.sync.dma_start(dbg_h1t[:, w * WCOL : (w + 1) * WCOL], st1[:])

            # ---- allgather -> table1 (128, R)
            nc.gpsimd.collective_compute(
                "AllGather",
                mybir.AluOpType.bypass,
                ins=[ag_in1.opt()],
                outs=[ag_out1.opt()],
                replica_groups=rg,
            )
            table = tabpool.tile([128, R], F32, tag="table")
            nc.sync.dma_start(table[:], ag_out1[:])
            if dbg:
                nc.sync.dma_start(dbg_tab[:], table[:])

            # ---- spmm layer -> dst_dram (nfeat, R) with act+bias per sub
            def spmm(table, sel, nfeat, biasap, act_fn, dst_dram, dbg_out=None):
                for k in range(SUBS):
                    L = int(glk[k])
                    o = int(offs[k])
                    gix = spool.tile([128, glkmax // 16], I16, tag="gix")
                    nc.sync.dma_start(gix[:, : L // 16], gidx[:, o // 16 : (o + L) // 16])
                    gath = spool.tile([128, glkmax], F32, tag="gath")
                    nc.gpsimd.ap_gather(
                        gath[:, :L].rearrange("c (n d) -> c n d", d=1),
                        table[:].rearrange("c (n d) -> c n d", d=1),
                        gix[:, : L // 16],
                        channels=128,
                        num_elems=R,
                        d=1,
                        num_idxs=L,
                    )
                    vr = spool.tile([128, glkmax], F32, tag="vr")
                    nc.sync.dma_start(vr[:, :L], valrep[:, o : o + L])
                    sc = spool.tile([128, glkmax], F32, tag="sc")
                    nc.vector.tensor_mul(sc[:, :L], gath[:, :L], vr[:, :L])
                    so = spool.tile([128, glkmax], F32, tag="so")
                    nc.vector.tensor_tensor_scan(
                        so[:, :L],
                        ones[:, :L],
                        sc[:, :L],
                        0.0,
                        mybir.AluOpType.mult,
                        mybir.AluOpType.add,
                    )
                    eix = spool.tile([128, EXT // 16], I16, tag="eix")
                    nc.sync.dma_start(
                        eix[:], eidx[:, k * EXT // 16 : (k + 1) * EXT // 16]
                    )
                    rxt = smpool.tile([128, EXT], F32, tag="rxt")
                    nc.gpsimd.ap_gather(
                        rxt[:].rearrange("c (n d) -> c n d", d=1),
                        so[:, :L].rearrange("c (n d) -> c n d", d=1),
                        eix[:],
                        channels=128,
                        num_elems=L,
                        d=1,
                        num_idxs=EXT,
                    )
                    dd = smpool.tile([128, EXT], F32, tag="dd")
                    nc.vector.tensor_sub(dd[:, 1:], rxt[:, 1:], rxt[:, : EXT - 1])
                    pb = ppool2.tile([16, cfg.SUBROWS], F32, tag="pb")
                    nc.tensor.matmul(
                        pb[:nfeat],
                        sel[:, :nfeat],
                        dd[:, 1 : 1 + cfg.SUBROWS],
                        start=True,
                        stop=True,
                    )
                    sto = smpool.tile([16, cfg.SUBROWS], F32, tag="sto")
                    nc.scalar.activation(sto[:nfeat], pb[:nfeat], act_fn, bias=biasap)
                    nc.sync.dma_start(
                        dst_dram[:, k * cfg.SUBROWS : (k + 1) * cfg.SUBROWS],
                        sto[:nfeat],
                    )
                    if dbg_out is not None:
                        nc.sync.dma_start(
                            dbg_out[:, k * cfg.SUBROWS : (k + 1) * cfg.SUBROWS],
                            sto[:nfeat],
                        )

            # ---- layer 1 spmm -> h_dram = relu(spmm1 + b1)
            spmm(table, sel1s, HID, b1s[:], mybir.ActivationFunctionType.Relu, h_dram,
                 dbg_out=dbg_h if dbg else None)

            # ---- z2t = W2^T h windows -> ag_in2 (rows OUT.. zeroed)
            for w in range(NW):
                hw_t = smpool.tile([HID, WCOL], F32, tag="hw")
                nc.sync.dma_start(hw_t[:], h_dram[:, w * WCOL : (w + 1) * WCOL])
                pz = ppool.tile([OUT, WCOL], F32, tag="pz")
                nc.tensor.matmul(pz[:], w2s[:], hw_t[:], start=True, stop=True)
                stz = smpool.tile([16, WCOL], F32, tag="stz")
                nc.vector.memset(stz[:], 0.0)
                nc.scalar.copy(stz[:OUT], pz[:])
                nc.sync.dma_start(ag_in2[:, w * WCOL : (w + 1) * WCOL], stz[:])

            # ---- allgather -> table2 (reuses table slot)
            nc.gpsimd.collective_compute(
                "AllGather",
                mybir.AluOpType.bypass,
                ins=[ag_in2.opt()],
                outs=[ag_out2.opt()],
                replica_groups=rg,
            )
            table2 = tabpool.tile([128, R], F32, tag="table")
            nc.sync.dma_start(table2[:], ag_out2[:])

            # ---- layer 2 spmm -> zc_dram = spmm2 + b2
            spmm(table2, sel2s, OUT, b2s[:], mybir.ActivationFunctionType.Identity,
                 zc_dram)

            # ---- log_softmax over OUT: transpose (OUT, R) -> row tiles
            zr = sfpool.tile([128, NRT, OUT], F32, tag="zr")
            for t in range(NRT):
                rp = 128 if t < NRT - 1 else RLAST
                zcs2 = smpool.tile([OUT, 128], F32, tag="zcs2")
                nc.sync.dma_start(zcs2[:, :rp], zc_dram[:, t * 128 : t * 128 + rp])
                pt = ppool2.tile([128, OUT], F32, tag="pt")
                nc.tensor.matmul(
                    pt[:rp],
                    zcs2[:, :rp],
                    idents[:OUT, :OUT],
                    is_transpose=True,
                    start=True,
                    stop=True,
                )
                nc.vector.tensor_copy(zr[:rp, t, :], pt[:rp])
            mx = sfpool.tile([128, NRT], F32, tag="mx")
            nc.vector.tensor_reduce(
                mx[:], zr[:], axis=mybir.AxisListType.X, op=mybir.AluOpType.max
            )
            zs = sfpool.tile([128, NRT, OUT], F32, tag="zs")
            for j in range(OUT):
                nc.vector.tensor_sub(zs[:, :, j], zr[:, :, j], mx[:])
            ex = sfpool.tile([128, NRT, OUT], F32, tag="ex")
            nc.scalar.activation(ex[:], zs[:], mybir.ActivationFunctionType.Exp)
            sm = sfpool.tile([128, NRT], F32, tag="sm")
            nc.vector.tensor_reduce(
                sm[:], ex[:], axis=mybir.AxisListType.X, op=mybir.AluOpType.add
            )
            lg = sfpool.tile([128, NRT], F32, tag="lg")
            nc.scalar.activation(lg[:], sm[:], mybir.ActivationFunctionType.Ln)
            for j in range(OUT):
                nc.vector.tensor_sub(zs[:, :, j], zs[:, :, j], lg[:])

            # ---- output DMA: zs (p, t, j) -> out rows 128t+p
            nc.sync.dma_start(
                out[: (NRT - 1) * 128, :].rearrange("(t p) j -> p t j", p=128),
                zs[:, : NRT - 1, :],
            )
            nc.sync.dma_start(
                out[(NRT - 1) * 128 :, :], zs[:RLAST, NRT - 1, :]
            )
    nc.compile()
    return nc


def kernel(x, adj_row, adj_col, adj_val, W1, b1, W2, b2):
    from concourse import bass_utils

    cfg = FULL
    in_maps, glk, offs = prepare(
        np.asarray(x), np.asarray(adj_row), np.asarray(adj_col),
        np.asarray(adj_val), np.asarray(W1), np.asarray(b1),
        np.asarray(W2), np.asarray(b2), cfg,
    )
    nc = build(cfg, glk, offs)
    res = bass_utils.run_bass_kernel_spmd(nc, in_maps, core_ids=list(range(cfg.C)))
    outs = [res.results[m]["out"] for m in range(cfg.C)]
    return np.concatenate(outs, axis=0)[: cfg.N]

